# revision 21
# baseline (speedup 1.0000x reference)
"""3-layer GCN on Trainium2, node-sharded across 8 NeuronCores.

Strategy (graph/data parallel, per sharding hint):
  - Nodes are sharded by contiguous range: core c owns dst rows
    [c*6272, (c+1)*6272) of a 50176-row padded node space.
  - Edges (incl. self-loops) are bucketed by dst window of 128 nodes and
    padded to a uniform T message-tiles per window so one SPMD program
    serves all cores (per-core data differs, program doesn't).
  - Symmetric norm dinv[src]*dinv[dst] is folded: table rows are
    pre-scaled by dinv (host for x, epilogue for hidden layers), and the
    dst factor is applied per-partition in the epilogue. The selection
    matrix is then a pure 0/1 one-hot, built with ONE tensor_tensor
    is_equal per window in dst-major layout sel2[m, d, t] (the broadcast
    access pattern keeps the innermost dim unit-stride so DVE runs in 2x
    mode); the aggregation matmul reads strided rhs slices sel2[:, :, t].
  - Layers 0/1: aggregate-then-transform — gather message rows from the
    replicated table (indirect DMA, fp8-e4m3 rows to halve the dominant
    gather traffic; pad slots skipped via bounds_check), accumulate on
    the PE into PSUM [feat, dst], dense matmul + bias + SiLU.
  - Layer 2 is transform-first: layer 1's tail transposes the scaled
    activations on the PE and multiplies by W2, so the layer-2 table is
    the [N, 64] Z = (dinv*silu) @ W2 — half the collective and gather
    bytes, and layer 2 needs no dense matmul (swapped-operand one-hot
    aggregation lands directly in [dst, class]).
  - Layer-boundary halo exchange = AllGather of the sharded activations
    (random graph => halo is essentially the full table).
  - log_softmax epilogue batches the Ln over all windows so the ACT
    engine loads each activation-function table once, not per window.
"""

import numpy as np

N = 50000
E = 800000
F = 128
H = 128
C = 64
NCORES = 8
WIN = 128
NWIN = 392          # ceil(N/128) rounded up to multiple of 8 -> 392 windows
NPAD = NWIN * WIN   # 50176
WPC = NWIN // NCORES  # 49 windows per core
NLOC = WPC * WIN      # 6272 rows per core

_CACHE = {}


def _prep_edges(edge_index):
    """Sort/bucket edges by dst window, pad to uniform tiles.

    Returns per-core arrays:
      idx [c, 128, WPC*T] int32   message source row (pad slots -> 0)
      dlc [c, 128, WPC*T] f32     dst slot within window (pad slots -> 200)
      dnv [c, 128, WPC]   f32     dinv of window dst nodes (col w = window w)
      dinv [N] f32                full per-node dinv (for host x prescale)
    """
    loop = np.arange(N, dtype=np.int64)
    src = np.concatenate([edge_index[0], loop]).astype(np.int32)
    dst = np.concatenate([edge_index[1], loop]).astype(np.int32)
    deg = np.bincount(dst, minlength=N).astype(np.float32)
    dinv = np.where(deg > 0, 1.0 / np.sqrt(np.maximum(deg, 1.0)), 0.0).astype(
        np.float32
    )

    order = np.argsort(dst, kind="stable")
    src, dst = src[order], dst[order]
    win = (dst >> 7).astype(np.int64)
    cnt = np.bincount(win, minlength=NWIN)
    T = int(-(-cnt.max() // WIN))

    starts = np.zeros(NWIN + 1, np.int64)
    starts[1:] = np.cumsum(cnt)
    slot = np.arange(len(src), dtype=np.int64) - starts[win]

    p_src = np.full((NWIN, T * WIN), 2**30, np.int32)
    p_dlc = np.full((NWIN, T * WIN), 200.0, np.float32)
    flat = win * (T * WIN) + slot
    p_src.ravel()[flat] = src
    p_dlc.ravel()[flat] = (dst & 127).astype(np.float32)

    def to_core_layout(a):
        # [NWIN, T*WIN] -> [c, w, t, p] -> [c, p, w, t] -> [c, 128, WPC*T]
        a = a.reshape(NCORES, WPC, T, WIN)
        return np.ascontiguousarray(a.transpose(0, 3, 1, 2).reshape(NCORES, WIN, WPC * T))

    dpad = np.zeros(NPAD, np.float32)
    dpad[:N] = dinv
    dnv = np.ascontiguousarray(
        dpad.reshape(NCORES, WPC, WIN).transpose(0, 2, 1)
    )  # [c, 128, WPC]

    return to_core_layout(p_src), to_core_layout(p_dlc), dnv, dinv, T


def _build_program(T, use_bf16=True, K=1, single_core=False, no_collectives=False, nch=1, skip_gather=False, skip_sel=False):
    from contextlib import ExitStack

    from concourse import bacc, bass, mybir, tile

    f32 = mybir.dt.float32
    bf16 = mybir.dt.bfloat16
    mdt = bf16 if use_bf16 else f32
    gdt = mybir.dt.float8e4 if use_bf16 else f32  # gather-table dtype (L0/L1)
    i32 = mybir.dt.int32
    eq = mybir.AluOpType.is_equal
    mul = mybir.AluOpType.mult
    add = mybir.AluOpType.add
    COLS = WPC * T

    nc = bacc.Bacc(
        "TRN2",
        target_bir_lowering=False,
        debug=False,
        enable_asserts=False,
        num_devices=1 if single_core else NCORES,
    )

    x_t = nc.dram_tensor("x", [NPAD, F], gdt, kind="ExternalInput")
    w_t = [
        nc.dram_tensor("w0", [F, H], mdt, kind="ExternalInput"),
        nc.dram_tensor("w1", [H, H], mdt, kind="ExternalInput"),
        nc.dram_tensor("w2", [H, C], mdt, kind="ExternalInput"),
    ]
    b_t = [
        nc.dram_tensor("b0", [WIN, H], f32, kind="ExternalInput"),
        nc.dram_tensor("b1", [WIN, H], f32, kind="ExternalInput"),
        nc.dram_tensor("b2", [WIN, C], f32, kind="ExternalInput"),
    ]
    idx_t = nc.dram_tensor("idx", [WIN, COLS], i32, kind="ExternalInput")
    dlc_t = nc.dram_tensor("dlc", [WIN, COLS], mdt, kind="ExternalInput")
    dnv_t = nc.dram_tensor("dnv", [WIN, WPC], f32, kind="ExternalInput")
    out_t = nc.dram_tensor("out", [NLOC, C], f32, kind="ExternalOutput")

    with tile.TileContext(nc) as tc, ExitStack() as ctx:
        dram = ctx.enter_context(tc.tile_pool(name="dram", bufs=1, space="DRAM"))
        tabspace = {} if single_core else {"addr_space": "Shared"}
        loc1s = [dram.tile([NLOC, H], gdt, name=f"loc1_{r}") for r in range(K)]
        loc2s = [dram.tile([NLOC, C], gdt, name=f"loc2_{r}") for r in range(K)]
        tab1s = [dram.tile([NPAD, H], gdt, name=f"tab1_{r}", **tabspace) for r in range(K)]
        tab2s = [dram.tile([NPAD, C], gdt, name=f"tab2_{r}", **tabspace) for r in range(K)]

        const = ctx.enter_context(tc.tile_pool(name="const", bufs=1))

        def load_const(name, src_t, shape, dtype):
            s = const.tile(shape, dtype, name=name)
            nc.sync.dma_start(out=s[:], in_=src_t[:])
            return s

        w_s = [
            load_const("w0s", w_t[0], [F, H], mdt),
            load_const("w1s", w_t[1], [H, H], mdt),
            load_const("w2s", w_t[2], [H, C], mdt),
        ]
        b_s = [
            load_const("b0s", b_t[0], [WIN, H], f32),
            load_const("b1s", b_t[1], [WIN, H], f32),
            load_const("b2s", b_t[2], [WIN, C], f32),
        ]
        idx_s = load_const("idxs", idx_t, [WIN, COLS], i32)
        dlc_s = load_const("dlcs", dlc_t, [WIN, COLS], mdt)
        dnv_s = load_const("dnvs", dnv_t, [WIN, WPC], f32)

        iota_i = const.tile([WIN, WIN], i32, name="iota_i")
        nc.gpsimd.iota(iota_i[:], pattern=[[1, WIN]], base=0, channel_multiplier=0)
        iota_b = const.tile([WIN, WIN], mdt, name="iota_b")
        nc.vector.tensor_copy(out=iota_b[:], in_=iota_i[:])
        # iota_rep[m, d, t] = d  (for the dst-major one-hot build)
        iota_rep = const.tile([WIN, WIN, T], mdt, name="iota_rep")
        nc.vector.tensor_copy(
            out=iota_rep[:], in_=iota_b[:].unsqueeze(2).broadcast_to([WIN, WIN, T])
        )
        iota_ci = const.tile([WIN, 1], i32, name="iota_ci")
        nc.gpsimd.iota(iota_ci[:], pattern=[[1, 1]], base=0, channel_multiplier=1)
        iota_cf = const.tile([WIN, 1], f32, name="iota_cf")
        nc.vector.tensor_copy(out=iota_cf[:], in_=iota_ci[:])
        ident = const.tile([WIN, WIN], mdt, name="ident")
        nc.vector.tensor_scalar(
            out=ident[:], in0=iota_b[:], scalar1=iota_cf[:, :1], scalar2=None, op0=eq
        )

        gpool = ctx.enter_context(tc.tile_pool(name="gp", bufs=6))
        selp = ctx.enter_context(tc.tile_pool(name="selp", bufs=5))
        epil = ctx.enter_context(tc.tile_pool(name="epil", bufs=8))
        soft = ctx.enter_context(tc.tile_pool(name="soft", bufs=1))
        psA = ctx.enter_context(tc.tile_pool(name="psA", bufs=3, space="PSUM"))
        psB = ctx.enter_context(tc.tile_pool(name="psB", bufs=2, space="PSUM"))
        psT = ctx.enter_context(tc.tile_pool(name="psT", bufs=1, space="PSUM"))

        for rep in range(K):
          loc1, loc2 = loc1s[rep], loc2s[rep]
          tab1, tab2 = tab1s[rep], tab2s[rep]
          tables = [x_t, tab1, tab2]
          for l in range(3):
            lr = f"{rep}_{l}"
            table = tables[l]
            Wl, bl = w_s[l], b_s[l]
            dest = [loc1, loc2, None][l]
            Fin = F if l < 2 else C
            if l == 2:
                lsb = soft.tile([WIN, WPC * C], f32, tag="lsb", name=f"lsb_{rep}")
                smb = soft.tile([WIN, WPC], f32, tag="smb", name=f"smb_{rep}")
                mxb = soft.tile([WIN, WPC], f32, tag="mxb", name=f"mxb_{rep}")
            for w in range(WPC):
                g = gpool.tile([WIN, T * Fin], gdt, tag="g", name=f"g_{lr}_{w}")
                if not skip_gather:
                    nc.gpsimd.indirect_dma_start(
                        out=g[:],
                        out_offset=None,
                        in_=table[:],
                        in_offset=bass.IndirectOffsetOnAxis(
                            ap=idx_s[:, w * T : (w + 1) * T], axis=0
                        ),
                        bounds_check=NPAD - 1,
                        oob_is_err=False,
                    )
                else:
                    nc.sync.dma_start(out=g[:, 0:Fin], in_=table[0:WIN, 0:Fin])
                # sel2[m, d, t] = 1 iff message m of tile t targets dst slot d
                sel2 = selp.tile([WIN, WIN, T], mdt, tag="sel", name=f"sel_{lr}_{w}")
                if not skip_sel:
                    nc.vector.tensor_tensor(
                        out=sel2[:],
                        in0=dlc_s[:, w * T : (w + 1) * T]
                        .unsqueeze(1)
                        .broadcast_to([WIN, WIN, T]),
                        in1=iota_rep[:],
                        op=eq,
                    )
                else:
                    nc.vector.tensor_copy(out=sel2[:, 0:2, :], in_=iota_rep[:, 0:2, :])
                dv = dnv_s[:, w : w + 1]
                if l < 2:
                    agg = psA.tile([F, WIN], f32, tag="agg", name=f"agg_{lr}_{w}")
                    for t in range(T):
                        # aggT[f, d] = sum_m msg[m, f] * sel[m, d]
                        nc.tensor.matmul(
                            out=agg[:],
                            lhsT=g[:, t * F : (t + 1) * F],
                            rhs=sel2[:, :, t],
                            start=(t == 0),
                            stop=(t == T - 1),
                        )
                    aggs = epil.tile([F, WIN], mdt, tag="aggs", name=f"aggs_{lr}_{w}")
                    nc.scalar.activation(
                        out=aggs[:], in_=agg[:],
                        func=mybir.ActivationFunctionType.Copy,
                    )
                    h_ps = psB.tile([WIN, H], f32, tag="h", name=f"h_{lr}_{w}")
                    nc.tensor.matmul(
                        out=h_ps[:], lhsT=aggs[:], rhs=Wl[:], start=True, stop=True
                    )
                    hb = epil.tile([WIN, H], f32, tag="hb", name=f"hb_{lr}_{w}")
                    nc.vector.scalar_tensor_tensor(
                        out=hb[:], in0=h_ps[:], scalar=dv, in1=bl[:], op0=mul, op1=add
                    )
                    act = epil.tile([WIN, H], mdt, tag="act", name=f"act_{lr}_{w}")
                    nc.scalar.activation(
                        out=act[:], in_=hb[:], func=mybir.ActivationFunctionType.Silu
                    )
                    acts = epil.tile(
                        [WIN, H], gdt if l == 0 else mdt, tag="acts",
                        name=f"acts_{lr}_{w}",
                    )
                    nc.vector.tensor_scalar_mul(acts[:], act[:], dv)
                    if l == 0:
                        nc.sync.dma_start(
                            out=dest[w * WIN : (w + 1) * WIN, :], in_=acts[:]
                        )
                    else:
                        # transform-first for layer 2: Z = (dinv*silu) @ W2,
                        # written as the (small) gather table for layer 2.
                        aT_ps = psT.tile([H, WIN], mdt, tag="aT", name=f"aT_{lr}_{w}")
                        nc.tensor.transpose(
                            out=aT_ps[:], in_=acts[:], identity=ident[:]
                        )
                        aT = epil.tile([H, WIN], mdt, tag="aTs", name=f"aTs_{lr}_{w}")
                        nc.scalar.activation(
                            out=aT[:], in_=aT_ps[:],
                            func=mybir.ActivationFunctionType.Copy,
                        )
                        z_ps = psB.tile([WIN, C], f32, tag="z", name=f"z_{lr}_{w}")
                        nc.tensor.matmul(
                            out=z_ps[:], lhsT=aT[:], rhs=w_s[2][:], start=True, stop=True
                        )
                        zst = epil.tile([WIN, C], gdt, tag="zst", name=f"zst_{lr}_{w}")
                        nc.scalar.activation(
                            out=zst[:], in_=z_ps[:],
                            func=mybir.ActivationFunctionType.Copy,
                        )
                        nc.sync.dma_start(
                            out=dest[w * WIN : (w + 1) * WIN, :], in_=zst[:]
                        )
                else:
                    agg = psA.tile([WIN, C], f32, tag="agg", name=f"agg_{lr}_{w}")
                    for t in range(T):
                        # agg[d, o] = sum_m sel[m, d] * z[m, o]
                        nc.tensor.matmul(
                            out=agg[:],
                            lhsT=sel2[:, :, t],
                            rhs=g[:, t * C : (t + 1) * C],
                            start=(t == 0),
                            stop=(t == T - 1),
                        )
                    lss = lsb[:, w * C : (w + 1) * C]
                    nc.vector.scalar_tensor_tensor(
                        out=lss, in0=agg[:], scalar=dv, in1=bl[:], op0=mul, op1=add
                    )
                    nc.vector.tensor_reduce(
                        out=mxb[:, w : w + 1],
                        in_=lss,
                        axis=mybir.AxisListType.X,
                        op=mybir.AluOpType.max,
                    )
                    nmx = epil.tile([WIN, 1], f32, tag="nmx", name=f"nmx_{rep}_{w}")
                    nc.vector.tensor_scalar_mul(nmx[:], mxb[:, w : w + 1], -1.0)
                    ex = epil.tile([WIN, C], f32, tag="ex", name=f"ex_{rep}_{w}")
                    nc.scalar.activation(
                        out=ex[:],
                        in_=lss,
                        func=mybir.ActivationFunctionType.Exp,
                        bias=nmx[:, :1],
                        scale=1.0,
                        accum_out=smb[:, w : w + 1],
                    )
            if l == 2:
                # batched log-sum-exp epilogue: one Ln for all windows
                lgb = soft.tile([WIN, WPC], f32, tag="lgb", name=f"lgb_{rep}")
                nc.scalar.activation(
                    out=lgb[:], in_=smb[:], func=mybir.ActivationFunctionType.Ln
                )
                lseb = soft.tile([WIN, WPC], f32, tag="lseb", name=f"lseb_{rep}")
                nc.vector.tensor_add(out=lseb[:], in0=lgb[:], in1=mxb[:])
                for w in range(WPC):
                    o = epil.tile([WIN, C], f32, tag="o", name=f"o_{rep}_{w}")
                    nc.vector.tensor_scalar_sub(
                        o[:], lsb[:, w * C : (w + 1) * C], lseb[:, w : w + 1]
                    )
                    nc.sync.dma_start(
                        out=out_t[w * WIN : (w + 1) * WIN, :], in_=o[:]
                    )
            if single_core or no_collectives or l == 2:
                pass
            else:
                loc, tab = (loc1, tab1) if l == 0 else (loc2, tab2)
                Ft = H if l == 0 else C
                # chunked AllGather: chunk i only waits on its producing
                # windows, so earlier chunks overlap the tail of this layer
                bnds = [round(i * WPC / nch) for i in range(nch + 1)]
                for i in range(nch):
                    r0, r1 = bnds[i] * WIN, bnds[i + 1] * WIN
                    if nch == 1:
                        outs_ap = tab.opt()
                    else:
                        outs_ap = (
                            tab[:]
                            .rearrange("(c r) h -> c r h", c=NCORES)[:, r0:r1, :]
                            .opt()
                        )
                    nc.gpsimd.collective_compute(
                        "AllGather",
                        mybir.AluOpType.bypass,
                        replica_groups=[list(range(NCORES))],
                        ins=[loc[r0:r1, :].opt()],
                        outs=[outs_ap],
                    )

    nc.compile()
    return nc


def _get_program(T):
    key = (T, USE_BF16)
    if key not in _CACHE:
        _CACHE[key] = _build_program(T, use_bf16=USE_BF16)
    return _CACHE[key]


USE_BF16 = True


def _make_in_maps(x, edge_index, W0, b0, W1, b1, W2, b2):
    import ml_dtypes

    mdt_np = ml_dtypes.bfloat16 if USE_BF16 else np.float32
    gdt_np = ml_dtypes.float8_e4m3 if USE_BF16 else np.float32
    x = np.asarray(x, np.float32)
    edge_index = np.asarray(edge_index)
    idx, dlc, dnv, dinv, T = _prep_edges(edge_index)

    x_pad = np.zeros((NPAD, F), gdt_np)
    x_pad[:N] = (x * dinv[:, None]).astype(gdt_np)
    common = {
        "x": x_pad,
        "w0": np.asarray(W0, np.float32).astype(mdt_np),
        "w1": np.asarray(W1, np.float32).astype(mdt_np),
        "w2": np.asarray(W2, np.float32).astype(mdt_np),
        "b0": np.broadcast_to(np.asarray(b0, np.float32), (WIN, H)).copy(),
        "b1": np.broadcast_to(np.asarray(b1, np.float32), (WIN, H)).copy(),
        "b2": np.broadcast_to(np.asarray(b2, np.float32), (WIN, C)).copy(),
    }
    in_maps = [
        dict(common, idx=idx[c], dlc=dlc[c].astype(mdt_np), dnv=dnv[c])
        for c in range(NCORES)
    ]
    return in_maps, T


def kernel(x, edge_index, W0, b0, W1, b1, W2, b2, **_):
    from concourse.bass_utils import run_bass_kernel_spmd

    in_maps, T = _make_in_maps(x, edge_index, W0, b0, W1, b1, W2, b2)
    nc = _get_program(T)
    res = run_bass_kernel_spmd(nc, in_maps, list(range(NCORES)))
    out = np.concatenate(
        [np.asarray(res.results[c]["out"]) for c in range(NCORES)], axis=0
    )
    return out[:N]


# revision 23
# speedup vs baseline: 1.7649x; 1.7649x over previous
"""3-layer GCN on Trainium2, node-sharded across 8 NeuronCores.

Strategy (graph/data parallel, per sharding hint):
  - Nodes are sharded by contiguous range: core c owns dst rows
    [c*6272, (c+1)*6272) of a 50176-row padded node space.
  - Edges (incl. self-loops) are bucketed by dst window of 128 nodes and
    padded to a uniform T message-tiles per window so one SPMD program
    serves all cores (per-core data differs, program doesn't).
  - Symmetric norm dinv[src]*dinv[dst] is folded: table rows are
    pre-scaled by dinv (host for x, epilogue for hidden layers), and the
    dst factor is applied per-partition in the epilogue. The selection
    matrix is then a pure 0/1 one-hot, built with ONE tensor_tensor
    is_equal per window in dst-major layout sel2[m, d, t] (the broadcast
    access pattern keeps the innermost dim unit-stride so DVE runs in 2x
    mode); the aggregation matmul reads strided rhs slices sel2[:, :, t].
  - Layers 0/1: aggregate-then-transform — gather message rows from the
    replicated table (indirect DMA, fp8-e4m3 rows to halve the dominant
    gather traffic; pad slots skipped via bounds_check), accumulate on
    the PE into PSUM [feat, dst], dense matmul + bias + SiLU.
  - Layer 2 is transform-first: layer 1's tail transposes the scaled
    activations on the PE and multiplies by W2, so the layer-2 table is
    the [N, 64] Z = (dinv*silu) @ W2 — half the collective and gather
    bytes, and layer 2 needs no dense matmul (swapped-operand one-hot
    aggregation lands directly in [dst, class]).
  - Layer-boundary halo exchange = AllGather of the sharded activations
    (random graph => halo is essentially the full table).
  - log_softmax epilogue batches the Ln over all windows so the ACT
    engine loads each activation-function table once, not per window.
"""

import numpy as np

N = 50000
E = 800000
F = 128
H = 128
C = 64
NCORES = 8
WIN = 128
NWIN = 392          # ceil(N/128) rounded up to multiple of 8 -> 392 windows
NPAD = NWIN * WIN   # 50176
WPC = NWIN // NCORES  # 49 windows per core
NLOC = WPC * WIN      # 6272 rows per core

_CACHE = {}


def _prep_edges(edge_index):
    """Sort/bucket edges by dst window, pad to uniform tiles.

    Returns per-core arrays:
      idx [c, 128, WPC*T] int32   message source row (pad slots -> 0)
      dlc [c, 128, WPC*T] f32     dst slot within window (pad slots -> 200)
      dnv [c, 128, WPC]   f32     dinv of window dst nodes (col w = window w)
      dinv [N] f32                full per-node dinv (for host x prescale)
    """
    loop = np.arange(N, dtype=np.int64)
    src = np.concatenate([edge_index[0], loop]).astype(np.int32)
    dst = np.concatenate([edge_index[1], loop]).astype(np.int32)
    deg = np.bincount(dst, minlength=N).astype(np.float32)
    dinv = np.where(deg > 0, 1.0 / np.sqrt(np.maximum(deg, 1.0)), 0.0).astype(
        np.float32
    )

    order = np.argsort(dst, kind="stable")
    src, dst = src[order], dst[order]
    win = (dst >> 7).astype(np.int64)
    cnt = np.bincount(win, minlength=NWIN)
    T = int(-(-cnt.max() // WIN))

    starts = np.zeros(NWIN + 1, np.int64)
    starts[1:] = np.cumsum(cnt)
    slot = np.arange(len(src), dtype=np.int64) - starts[win]

    p_src = np.full((NWIN, T * WIN), 2**30, np.int32)
    p_dlc = np.full((NWIN, T * WIN), 200.0, np.float32)
    flat = win * (T * WIN) + slot
    p_src.ravel()[flat] = src
    p_dlc.ravel()[flat] = (dst & 127).astype(np.float32)

    def to_core_layout(a):
        # [NWIN, T*WIN] -> [c, w, t, p] -> [c, p, w, t] -> [c, 128, WPC*T]
        a = a.reshape(NCORES, WPC, T, WIN)
        return np.ascontiguousarray(a.transpose(0, 3, 1, 2).reshape(NCORES, WIN, WPC * T))

    dpad = np.zeros(NPAD, np.float32)
    dpad[:N] = dinv
    dnv = np.ascontiguousarray(
        dpad.reshape(NCORES, WPC, WIN).transpose(0, 2, 1)
    )  # [c, 128, WPC]

    return to_core_layout(p_src), to_core_layout(p_dlc), dnv, dinv, T


def _build_program(T, use_bf16=True, K=1, single_core=False, no_collectives=False, nch=1, skip_gather=False, skip_sel=False):
    from contextlib import ExitStack

    from concourse import bacc, bass, mybir, tile

    f32 = mybir.dt.float32
    bf16 = mybir.dt.bfloat16
    mdt = bf16 if use_bf16 else f32
    gdt = mybir.dt.float8e4 if use_bf16 else f32  # gather-table dtype (L0/L1)
    i32 = mybir.dt.int32
    eq = mybir.AluOpType.is_equal
    mul = mybir.AluOpType.mult
    add = mybir.AluOpType.add
    COLS = WPC * T

    nc = bacc.Bacc(
        "TRN2",
        target_bir_lowering=False,
        debug=False,
        enable_asserts=False,
        num_devices=1 if single_core else NCORES,
    )

    x_t = nc.dram_tensor("x", [NPAD, F], gdt, kind="ExternalInput")
    w_t = [
        nc.dram_tensor("w0", [F, H], mdt, kind="ExternalInput"),
        nc.dram_tensor("w1", [H, H], mdt, kind="ExternalInput"),
        nc.dram_tensor("w2", [H, C], mdt, kind="ExternalInput"),
    ]
    b_t = [
        nc.dram_tensor("b0", [WIN, H], f32, kind="ExternalInput"),
        nc.dram_tensor("b1", [WIN, H], f32, kind="ExternalInput"),
        nc.dram_tensor("b2", [WIN, C], f32, kind="ExternalInput"),
    ]
    idx_t = nc.dram_tensor("idx", [WIN, COLS], i32, kind="ExternalInput")
    dlc_t = nc.dram_tensor("dlc", [WIN, COLS], mdt, kind="ExternalInput")
    dnv_t = nc.dram_tensor("dnv", [WIN, WPC], f32, kind="ExternalInput")
    out_t = nc.dram_tensor("out", [NLOC, C], f32, kind="ExternalOutput")

    with tile.TileContext(nc) as tc, ExitStack() as ctx:
        dram = ctx.enter_context(tc.tile_pool(name="dram", bufs=1, space="DRAM"))
        tabspace = {} if single_core else {"addr_space": "Shared"}
        loc1s = [dram.tile([NLOC, H], gdt, name=f"loc1_{r}") for r in range(K)]
        loc2s = [dram.tile([NLOC, C], gdt, name=f"loc2_{r}") for r in range(K)]
        tab1s = [dram.tile([NPAD, H], gdt, name=f"tab1_{r}", **tabspace) for r in range(K)]
        tab2s = [dram.tile([NPAD, C], gdt, name=f"tab2_{r}", **tabspace) for r in range(K)]

        const = ctx.enter_context(tc.tile_pool(name="const", bufs=1))

        def load_const(name, src_t, shape, dtype):
            s = const.tile(shape, dtype, name=name)
            nc.sync.dma_start(out=s[:], in_=src_t[:])
            return s

        w_s = [
            load_const("w0s", w_t[0], [F, H], mdt),
            load_const("w1s", w_t[1], [H, H], mdt),
            load_const("w2s", w_t[2], [H, C], mdt),
        ]
        b_s = [
            load_const("b0s", b_t[0], [WIN, H], f32),
            load_const("b1s", b_t[1], [WIN, H], f32),
            load_const("b2s", b_t[2], [WIN, C], f32),
        ]
        idx_s = load_const("idxs", idx_t, [WIN, COLS], i32)
        dlc_s = load_const("dlcs", dlc_t, [WIN, COLS], mdt)
        dnv_s = load_const("dnvs", dnv_t, [WIN, WPC], f32)

        iota_i = const.tile([WIN, WIN], i32, name="iota_i")
        nc.gpsimd.iota(iota_i[:], pattern=[[1, WIN]], base=0, channel_multiplier=0)
        iota_b = const.tile([WIN, WIN], mdt, name="iota_b")
        nc.vector.tensor_copy(out=iota_b[:], in_=iota_i[:])
        # iota_rep[m, d, t] = d  (for the dst-major one-hot build)
        iota_rep = const.tile([WIN, WIN, T], mdt, name="iota_rep")
        nc.vector.tensor_copy(
            out=iota_rep[:], in_=iota_b[:].unsqueeze(2).broadcast_to([WIN, WIN, T])
        )
        iota_ci = const.tile([WIN, 1], i32, name="iota_ci")
        nc.gpsimd.iota(iota_ci[:], pattern=[[1, 1]], base=0, channel_multiplier=1)
        iota_cf = const.tile([WIN, 1], f32, name="iota_cf")
        nc.vector.tensor_copy(out=iota_cf[:], in_=iota_ci[:])
        ident = const.tile([WIN, WIN], mdt, name="ident")
        nc.vector.tensor_scalar(
            out=ident[:], in0=iota_b[:], scalar1=iota_cf[:, :1], scalar2=None, op0=eq
        )

        gpool = ctx.enter_context(tc.tile_pool(name="gp", bufs=4))
        selp = ctx.enter_context(tc.tile_pool(name="selp", bufs=4))
        epil = ctx.enter_context(tc.tile_pool(name="epil", bufs=8))
        soft = ctx.enter_context(tc.tile_pool(name="soft", bufs=1))
        selc = ctx.enter_context(tc.tile_pool(name="selc", bufs=1))
        SELC = 20  # windows whose one-hot is cached in SBUF across layers
        psA = ctx.enter_context(tc.tile_pool(name="psA", bufs=3, space="PSUM"))
        psB = ctx.enter_context(tc.tile_pool(name="psB", bufs=2, space="PSUM"))
        psT = ctx.enter_context(tc.tile_pool(name="psT", bufs=1, space="PSUM"))

        for rep in range(K):
          loc1, loc2 = loc1s[rep], loc2s[rep]
          tab1, tab2 = tab1s[rep], tab2s[rep]
          tables = [x_t, tab1, tab2]
          sel_cache = {}
          for l in range(3):
            lr = f"{rep}_{l}"
            table = tables[l]
            Wl, bl = w_s[l], b_s[l]
            dest = [loc1, loc2, None][l]
            Fin = F if l < 2 else C
            if l == 2:
                lsb = soft.tile([WIN, WPC * C], f32, tag="lsb", name=f"lsb_{rep}")
                smb = soft.tile([WIN, WPC], f32, tag="smb", name=f"smb_{rep}")
                mxb = soft.tile([WIN, WPC], f32, tag="mxb", name=f"mxb_{rep}")
            for w in range(WPC):
                g = gpool.tile([WIN, T * Fin], gdt, tag="g", name=f"g_{lr}_{w}")
                if not skip_gather:
                    nc.gpsimd.indirect_dma_start(
                        out=g[:],
                        out_offset=None,
                        in_=table[:],
                        in_offset=bass.IndirectOffsetOnAxis(
                            ap=idx_s[:, w * T : (w + 1) * T], axis=0
                        ),
                        bounds_check=NPAD - 1,
                        oob_is_err=False,
                    )
                else:
                    nc.sync.dma_start(out=g[:, 0:Fin], in_=table[0:WIN, 0:Fin])
                # sel2[m, d, t] = 1 iff message m of tile t targets dst slot d
                if w < SELC and l > 0:
                    sel2 = sel_cache[w]   # graph-constant one-hot, built in l=0
                else:
                    if w < SELC:
                        sel2 = selc.tile(
                            [WIN, WIN, T], mdt, tag=f"sc{w}", name=f"sc_{rep}_{w}"
                        )
                        sel_cache[w] = sel2
                    else:
                        sel2 = selp.tile(
                            [WIN, WIN, T], mdt, tag="sel", name=f"sel_{lr}_{w}"
                        )
                    if not skip_sel:
                        nc.vector.tensor_tensor(
                            out=sel2[:],
                            in0=dlc_s[:, w * T : (w + 1) * T]
                            .unsqueeze(1)
                            .broadcast_to([WIN, WIN, T]),
                            in1=iota_rep[:],
                            op=eq,
                        )
                    else:
                        nc.vector.tensor_copy(
                            out=sel2[:, 0:2, :], in_=iota_rep[:, 0:2, :]
                        )
                dv = dnv_s[:, w : w + 1]
                if l < 2:
                    agg = psA.tile([F, WIN], f32, tag="agg", name=f"agg_{lr}_{w}")
                    for t in range(T):
                        # aggT[f, d] = sum_m msg[m, f] * sel[m, d]
                        nc.tensor.matmul(
                            out=agg[:],
                            lhsT=g[:, t * F : (t + 1) * F],
                            rhs=sel2[:, :, t],
                            start=(t == 0),
                            stop=(t == T - 1),
                        )
                    aggs = epil.tile([F, WIN], mdt, tag="aggs", name=f"aggs_{lr}_{w}")
                    nc.vector.tensor_copy(out=aggs[:], in_=agg[:])
                    h_ps = psB.tile([WIN, H], f32, tag="h", name=f"h_{lr}_{w}")
                    nc.tensor.matmul(
                        out=h_ps[:], lhsT=aggs[:], rhs=Wl[:], start=True, stop=True
                    )
                    hb = epil.tile([WIN, H], f32, tag="hb", name=f"hb_{lr}_{w}")
                    nc.vector.scalar_tensor_tensor(
                        out=hb[:], in0=h_ps[:], scalar=dv, in1=bl[:], op0=mul, op1=add
                    )
                    act = epil.tile([WIN, H], mdt, tag="act", name=f"act_{lr}_{w}")
                    nc.scalar.activation(
                        out=act[:], in_=hb[:], func=mybir.ActivationFunctionType.Silu
                    )
                    acts = epil.tile(
                        [WIN, H], gdt if l == 0 else mdt, tag="acts",
                        name=f"acts_{lr}_{w}",
                    )
                    nc.vector.tensor_scalar_mul(acts[:], act[:], dv)
                    if l == 0:
                        nc.sync.dma_start(
                            out=dest[w * WIN : (w + 1) * WIN, :], in_=acts[:]
                        )
                    else:
                        # transform-first for layer 2: Z = (dinv*silu) @ W2,
                        # written as the (small) gather table for layer 2.
                        aT_ps = psT.tile([H, WIN], mdt, tag="aT", name=f"aT_{lr}_{w}")
                        nc.tensor.transpose(
                            out=aT_ps[:], in_=acts[:], identity=ident[:]
                        )
                        aT = epil.tile([H, WIN], mdt, tag="aTs", name=f"aTs_{lr}_{w}")
                        nc.vector.tensor_copy(out=aT[:], in_=aT_ps[:])
                        z_ps = psB.tile([WIN, C], f32, tag="z", name=f"z_{lr}_{w}")
                        nc.tensor.matmul(
                            out=z_ps[:], lhsT=aT[:], rhs=w_s[2][:], start=True, stop=True
                        )
                        zst = epil.tile([WIN, C], gdt, tag="zst", name=f"zst_{lr}_{w}")
                        nc.vector.tensor_copy(out=zst[:], in_=z_ps[:])
                        nc.sync.dma_start(
                            out=dest[w * WIN : (w + 1) * WIN, :], in_=zst[:]
                        )
                else:
                    agg = psA.tile([WIN, C], f32, tag="agg", name=f"agg_{lr}_{w}")
                    for t in range(T):
                        # agg[d, o] = sum_m sel[m, d] * z[m, o]
                        nc.tensor.matmul(
                            out=agg[:],
                            lhsT=sel2[:, :, t],
                            rhs=g[:, t * C : (t + 1) * C],
                            start=(t == 0),
                            stop=(t == T - 1),
                        )
                    lss = lsb[:, w * C : (w + 1) * C]
                    nc.vector.scalar_tensor_tensor(
                        out=lss, in0=agg[:], scalar=dv, in1=bl[:], op0=mul, op1=add
                    )
                    nc.vector.tensor_reduce(
                        out=mxb[:, w : w + 1],
                        in_=lss,
                        axis=mybir.AxisListType.X,
                        op=mybir.AluOpType.max,
                    )
                    nmx = epil.tile([WIN, 1], f32, tag="nmx", name=f"nmx_{rep}_{w}")
                    nc.vector.tensor_scalar_mul(nmx[:], mxb[:, w : w + 1], -1.0)
                    ex = epil.tile([WIN, C], f32, tag="ex", name=f"ex_{rep}_{w}")
                    nc.scalar.activation(
                        out=ex[:],
                        in_=lss,
                        func=mybir.ActivationFunctionType.Exp,
                        bias=nmx[:, :1],
                        scale=1.0,
                        accum_out=smb[:, w : w + 1],
                    )
            if l == 2:
                # batched log-sum-exp epilogue: one Ln for all windows
                lgb = soft.tile([WIN, WPC], f32, tag="lgb", name=f"lgb_{rep}")
                nc.scalar.activation(
                    out=lgb[:], in_=smb[:], func=mybir.ActivationFunctionType.Ln
                )
                lseb = soft.tile([WIN, WPC], f32, tag="lseb", name=f"lseb_{rep}")
                nc.vector.tensor_add(out=lseb[:], in0=lgb[:], in1=mxb[:])
                for w in range(WPC):
                    o = epil.tile([WIN, C], f32, tag="o", name=f"o_{rep}_{w}")
                    nc.vector.tensor_scalar_sub(
                        o[:], lsb[:, w * C : (w + 1) * C], lseb[:, w : w + 1]
                    )
                    nc.sync.dma_start(
                        out=out_t[w * WIN : (w + 1) * WIN, :], in_=o[:]
                    )
            if single_core or no_collectives or l == 2:
                pass
            else:
                loc, tab = (loc1, tab1) if l == 0 else (loc2, tab2)
                Ft = H if l == 0 else C
                # chunked AllGather: chunk i only waits on its producing
                # windows, so earlier chunks overlap the tail of this layer
                bnds = [round(i * WPC / nch) for i in range(nch + 1)]
                for i in range(nch):
                    r0, r1 = bnds[i] * WIN, bnds[i + 1] * WIN
                    if nch == 1:
                        outs_ap = tab.opt()
                    else:
                        outs_ap = (
                            tab[:]
                            .rearrange("(c r) h -> c r h", c=NCORES)[:, r0:r1, :]
                            .opt()
                        )
                    nc.gpsimd.collective_compute(
                        "AllGather",
                        mybir.AluOpType.bypass,
                        replica_groups=[list(range(NCORES))],
                        ins=[loc[r0:r1, :].opt()],
                        outs=[outs_ap],
                    )

    nc.compile()
    return nc


def _get_program(T):
    key = (T, USE_BF16)
    if key not in _CACHE:
        _CACHE[key] = _build_program(T, use_bf16=USE_BF16)
    return _CACHE[key]


USE_BF16 = True


def _make_in_maps(x, edge_index, W0, b0, W1, b1, W2, b2):
    import ml_dtypes

    mdt_np = ml_dtypes.bfloat16 if USE_BF16 else np.float32
    gdt_np = ml_dtypes.float8_e4m3 if USE_BF16 else np.float32
    x = np.asarray(x, np.float32)
    edge_index = np.asarray(edge_index)
    idx, dlc, dnv, dinv, T = _prep_edges(edge_index)

    x_pad = np.zeros((NPAD, F), gdt_np)
    x_pad[:N] = (x * dinv[:, None]).astype(gdt_np)
    common = {
        "x": x_pad,
        "w0": np.asarray(W0, np.float32).astype(mdt_np),
        "w1": np.asarray(W1, np.float32).astype(mdt_np),
        "w2": np.asarray(W2, np.float32).astype(mdt_np),
        "b0": np.broadcast_to(np.asarray(b0, np.float32), (WIN, H)).copy(),
        "b1": np.broadcast_to(np.asarray(b1, np.float32), (WIN, H)).copy(),
        "b2": np.broadcast_to(np.asarray(b2, np.float32), (WIN, C)).copy(),
    }
    in_maps = [
        dict(common, idx=idx[c], dlc=dlc[c].astype(mdt_np), dnv=dnv[c])
        for c in range(NCORES)
    ]
    return in_maps, T


def kernel(x, edge_index, W0, b0, W1, b1, W2, b2, **_):
    from concourse.bass_utils import run_bass_kernel_spmd

    in_maps, T = _make_in_maps(x, edge_index, W0, b0, W1, b1, W2, b2)
    nc = _get_program(T)
    res = run_bass_kernel_spmd(nc, in_maps, list(range(NCORES)))
    out = np.concatenate(
        [np.asarray(res.results[c]["out"]) for c in range(NCORES)], axis=0
    )
    return out[:N]
